# revision 63
# baseline (speedup 1.0000x reference)
"""Bass/Tile TRN2 kernel for bilinear-score attention (score softmax + context).

reference:
    qW     = query @ W                      [B, Tq, Dk]
    weight = qW @ keys^T + mask[:, None, :] [B, Tq, Tk]
    score  = softmax(weight, axis=-1)
    ctx    = score @ values                 [B, Tq, Dv]
    returns (score, ctx)

Sharding: data-parallel over batch B=16 across 8 NeuronCores (2 batches/core).
Numerics: fp16 hi/lo 3-pass matmuls (hh + hl + lh) for both big contractions
(near-fp32 logits; exact fp32 mask added during the PSUM drain); phase 3 is a
single all-fp16 pass (softmax scores and values cast to fp16, values cast
in-flight by the SWDGE DMA). Measured vs fp32 reference: score rel err ~4e-5,
ctx rel err ~5e-4. Per-core device time ~439 us (cost-model timeline, ~93% PE occupancy;
corroborated by an isolated-cache in-NEFF-repetition wall-clock slope).
"""

import os
import sys

import numpy as np

os.environ.setdefault("JAX_COMPILATION_CACHE_DIR", "/tmp/jax_comp_cache")

for _p in ("/opt/trn_rl_repo",):
    if _p not in sys.path and os.path.isdir(_p):
        sys.path.insert(0, _p)

import concourse.bass as bass  # noqa: E402
import concourse.tile as tile  # noqa: E402
from concourse import bacc, mybir  # noqa: E402
from concourse.bass import ds, ts  # noqa: E402
from concourse.bass_utils import run_bass_kernel_spmd  # noqa: E402

import json as _json

OPTS = {
    "drain_copy": True,      # early PSUM->SBUF drain, softmax reads copy
    "vals_swdge": False,     # values load via gpsimd SWDGE
    "stage_hi": "act",       # engine for hi cast: act|dve|gpsimd
    "stage_lo": "dve",       # engine for lo subtract: dve|gpsimd
    "stage_ring": "sync",    # ring for staging dma: sync|scalar
    "order": "pipelined",    # pipelined|serial
    "interleave_p3b1": True,
    "interleave_p3b0": False,
    "interleave_both": True,
    "score_t": "perqt",      # perqt|batched
    "out_ring": "sync",      # ring for score/ctx/s16 outputs
    "pair_stage": False,     # batch staging loads/writes in pairs
    "s16_ring": "gpsimd",
    "load_ring": "sync",
    "il_offset": 4,
    "stagew_split": False,
    "p3b0_in_p1b1": True,
    "q0_pe_t": True,
    "k_floor_ms": 0.03,
}
if os.environ.get("K_OPTS"):
    OPTS.update(_json.loads(os.environ["K_OPTS"]))

P = 128
T = 1024
NT = T // P  # 8
NB = 2       # batches per core
NCORES = 8
F32 = mybir.dt.float32
F16 = mybir.dt.float16
F32R = mybir.dt.float32r
AX = mybir.AxisListType
AOP = mybir.AluOpType
AF = mybir.ActivationFunctionType


def _eng(nc, name):
    return {"act": nc.scalar, "dve": nc.vector, "gpsimd": nc.gpsimd}[name]


def _ring(nc, name):
    return {"sync": nc.sync, "scalar": nc.scalar, "gpsimd": nc.gpsimd}[name]


def _hi_lo_to_scratch(nc, stage, src_ap, hi_scr, lo_scr):
    """Load fp32 rows, split into fp16 hi/lo, store to DRAM scratch."""
    ring = _ring(nc, OPTS["stage_ring"])
    hi_e = _eng(nc, OPTS["stage_hi"])
    lo_e = _eng(nc, OPTS["stage_lo"])
    if not OPTS["pair_stage"]:
        for t in range(NT):
            xf = stage.tile([P, T], F32, tag="ldf32", bufs=4)
            _ring(nc, OPTS["load_ring"]).dma_start(xf[:], src_ap[ts(t, P), :])
            xh = stage.tile([P, T], F16, tag="hi16", bufs=4)
            if hi_e is nc.scalar:
                nc.scalar.copy(xh[:], xf[:])
            else:
                hi_e.tensor_copy(xh[:], xf[:])
            xl = stage.tile([P, T], F16, tag="lo16")
            lo_e.tensor_tensor(xl[:], xf[:], xh[:], AOP.subtract)
            ring.dma_start(hi_scr[ts(t, P), :], xh[:])
            ring.dma_start(lo_scr[ts(t, P), :], xl[:])
        return
    for t2 in range(NT // 2):
        xf = stage.tile([P, 2, T], F32, tag="ldf32pair", bufs=1)
        ring.dma_start(xf[:], src_ap[ds(t2 * 2 * P, 2 * P), :].rearrange("(o p) e -> p o e", p=P))
        xh = stage.tile([P, 2, T], F16, tag="hi16")
        xl = stage.tile([P, 2, T], F16, tag="lo16")
        for j in range(2):
            if hi_e is nc.scalar:
                nc.scalar.copy(xh[:, j, :], xf[:, j, :])
            else:
                hi_e.tensor_copy(xh[:, j, :], xf[:, j, :])
            lo_e.tensor_tensor(xl[:, j, :], xf[:, j, :], xh[:, j, :], AOP.subtract)
        ring.dma_start(
            hi_scr[ds(t2 * 2 * P, 2 * P), :].rearrange("(o p) e -> p o e", p=P), xh[:]
        )
        ring.dma_start(
            lo_scr[ds(t2 * 2 * P, 2 * P), :].rearrange("(o p) e -> p o e", p=P), xl[:]
        )


def _phase1(nc, pools, b, W_hi, W_lo, qTh, qTl, interleave=None):
    """qWT[e, q] = W^T @ query^T as fp16 hi/lo, 3-pass per psum tile."""
    qwt_pool, psA = pools["qwt"], pools["psA"]
    qWTh = qwt_pool.tile([P, NT, T], F16, tag="qWTh")
    qWTl = qwt_pool.tile([P, NT, T], F16, tag="qWTl")
    for et in range(NT):
        if interleave is not None:
            interleave(et)
        ps = psA.tile([P, T], F32, tag="psA")
        for dt_ in range(NT):
            lw_h = W_hi[:, dt_, ts(et, P)]
            lw_l = W_lo[:, dt_, ts(et, P)]
            first = dt_ == 0
            last = dt_ == NT - 1
            for qc in range(2):
                nc.tensor.matmul(
                    ps[:, ds(qc * 512, 512)], lw_h,
                    qTh[:, dt_, ds(qc * 512, 512)], start=first, stop=False,
                )
            for qc in range(2):
                nc.tensor.matmul(
                    ps[:, ds(qc * 512, 512)], lw_h,
                    qTl[:, dt_, ds(qc * 512, 512)], start=False, stop=False,
                )
            for qc in range(2):
                nc.tensor.matmul(
                    ps[:, ds(qc * 512, 512)], lw_l,
                    qTh[:, dt_, ds(qc * 512, 512)], start=False, stop=last,
                )
        nc.scalar.copy(qWTh[:, et, :], ps[:])
        nc.vector.tensor_tensor(qWTl[:, et, :], ps[:], qWTh[:, et, :], AOP.subtract)
    return qWTh, qWTl


def _phase2_softmax(nc, pools, b, s_d, qWTh, qWTl, kTh, kTl, ones, mrep, s16_scr,
                    interleave=None, skip_tail=False, off=None):
    """weight[q, k] = qW @ keys^T + mask; softmax rows; write score + fp16 copy."""
    soft, sc_pool, psB = pools["soft"], pools["sc"], pools["psB"]
    if off is None:
        off = OPTS["il_offset"]
    for qt_ in range(NT):
        if interleave is not None and qt_ >= off:
            interleave(qt_ - off, False)
        ps2 = psB.tile([P, T], F32, tag="psB")
        for et in range(NT):
            for li, (lhs, rhs) in enumerate(((qWTh, kTh), (qWTh, kTl), (qWTl, kTh))):
                lw = lhs[:, et, ts(qt_, P)]
                for kc in range(2):
                    nc.tensor.matmul(
                        ps2[:, ds(kc * 512, 512)], lw, rhs[:, et, ds(kc * 512, 512)],
                        start=(et == 0 and li == 0),
                        stop=(et == NT - 1 and li == 2),
                    )
        # drain + mask add fused: wsb = ps2 + mask (fp32, exact)
        wsb = soft.tile([P, T], F32, tag="wsb")
        nc.vector.tensor_tensor(wsb[:], ps2[:], mrep[:], AOP.add)
        negmax = soft.tile([P, 1], F32, tag="negmax")
        nc.vector.tensor_reduce(negmax[:], wsb[:], axis=AX.X, op=AOP.max, negate=True)
        expt = soft.tile([P, T], F32, tag="expt")
        sumexp = soft.tile([P, 1], F32, tag="sumexp")
        nc.scalar.activation(
            expt[:], wsb[:], AF.Exp, bias=negmax[:], scale=1.0, accum_out=sumexp[:]
        )
        recip = soft.tile([P, 1], F32, tag="recip")
        nc.vector.reciprocal(recip[:], sumexp[:])
        nc.vector.tensor_scalar_mul(expt[:], expt[:], recip[:])
        _ring(nc, OPTS["out_ring"]).dma_start(s_d[b, ts(qt_, P), :], expt[:])
        s16 = sc_pool.tile([P, T], F16, tag="s16t")
        pools["s16_tiles"][(b, qt_)] = s16
        nc.scalar.copy(s16[:], expt[:])
        _ring(nc, OPTS["s16_ring"]).dma_start(s16_scr[ts(qt_, P), :], s16[:])
    if interleave is not None and not skip_tail:
        for qt_ in range(NT - off, NT):
            interleave(qt_, True)


def _phase3_qt_pe(nc, pools, b, c_d, s16_tile, vals, qt_, ps_pool="psB"):
    """Tail variant: transpose score on the (idle) PE from the resident s16
    SBUF tile via an fp16 PSUM bitcast view; skips the scratch round-trip."""
    st_pool, cx_pool = pools["st"], pools["cx"]
    ident16 = pools["ident16"]
    ps = pools[ps_pool].tile([P, T], F32, tag=ps_pool)
    view = ps[:, ds(0, 512)].bitcast(F16)
    for kt_ in range(NT):
        nc.tensor.transpose(view[:, ts(kt_, P)], s16_tile[:, ts(kt_, P)], ident16)
    sT16 = st_pool.tile([P, NT, P], F16, tag="sT16")
    nc.scalar.copy(sT16[:], view.rearrange("p (o q) -> p o q", q=P))
    ps3 = ps[:, ds(512, 512)]
    # two vc halves accumulate sequentially into the same psum half (bank 1)
    cxt = cx_pool.tile([P, T], F32, tag="cx")
    for vc in range(2):
        for kt_ in range(NT):
            nc.tensor.matmul(
                ps3, sT16[:, kt_, :], vals[:, kt_, ds(vc * 512, 512)],
                start=(kt_ == 0), stop=(kt_ == NT - 1),
            )
        nc.scalar.copy(cxt[:, ds(vc * 512, 512)], ps3)
    _ring(nc, OPTS["out_ring"]).dma_start(c_d[b, ts(qt_, P), :], cxt[:])


def _phase3_qt(nc, pools, b, c_d, s16_scr, vals, qt_, ps_pool="psA"):
    """ctx[qt block] = score @ values, all fp16, one pass."""
    st_pool, cx_pool = pools["st"], pools["cx"]
    sT16 = st_pool.tile([P, NT, P], F16, tag="sT16")
    for kt_ in range(NT):
        nc.sync.dma_start_transpose(sT16[:, kt_, :], s16_scr[ts(qt_, P), ts(kt_, P)])
    ps3 = pools[ps_pool].tile([P, T], F32, tag=ps_pool)
    for kt_ in range(NT):
        lw = sT16[:, kt_, :]
        for vc in range(2):
            nc.tensor.matmul(
                ps3[:, ds(vc * 512, 512)], lw, vals[:, kt_, ds(vc * 512, 512)],
                start=(kt_ == 0), stop=(kt_ == NT - 1),
            )
    cx = cx_pool.tile([P, T], F32, tag="cx")
    nc.scalar.copy(cx[:], ps3[:])
    _ring(nc, OPTS["out_ring"]).dma_start(c_d[b, ts(qt_, P), :], cx[:])


def _phase3(nc, pools, b, c_d, s16_scr, vals):
    if OPTS["score_t"] == "perqt":
        for qt_ in range(NT):
            _phase3_qt(nc, pools, b, c_d, s16_scr, vals, qt_)
        return
    st_pool, cx_pool, psA = pools["st"], pools["cx"], pools["psA"]
    sTall = st_pool.tile([P, NT, T], F16, tag="sTall", bufs=1)
    for kt_ in range(NT):
        nc.sync.dma_start_transpose(sTall[:, kt_, :], s16_scr[:, ts(kt_, P)])
    for qt_ in range(NT):
        ps3 = psA.tile([P, T], F32, tag="psA")
        for kt_ in range(NT):
            lw = sTall[:, kt_, ts(qt_, P)]
            for vc in range(2):
                nc.tensor.matmul(
                    ps3[:, ds(vc * 512, 512)], lw, vals[:, kt_, ds(vc * 512, 512)],
                    start=(kt_ == 0), stop=(kt_ == NT - 1),
                )
        cx = cx_pool.tile([P, T], F32, tag="cx")
        nc.scalar.copy(cx[:], ps3[:])
        _ring(nc, OPTS["out_ring"]).dma_start(c_d[b, ts(qt_, P), :], cx[:])


def _stage_q_pe(nc, pools, b, q_d, ident):
    """Transpose query on the (idle) PE: fp32 tile transposes into PSUM, then
    one fused drain per q-tile into the fp16 hi/lo qT layout. No DRAM
    round-trip, no xbar."""
    stage, qt_pool, psB = pools["stage"], pools["qt"], pools["psB"]
    qTh = qt_pool.tile([P, NT, T], F16, tag="qTh")
    qTl = qt_pool.tile([P, NT, T], F16, tag="qTl")
    for qt_ in range(NT):
        xf = stage.tile([P, T], F32, tag="ldf32", bufs=4)
        _ring(nc, OPTS["load_ring"]).dma_start(xf[:], q_d[b, ts(qt_, P), :])
        pst = psB.tile([P, T], F32, tag="psB")
        for dt_ in range(NT):
            nc.tensor.transpose(pst[:, ts(dt_, P)], xf[:, ts(dt_, P)], ident)
        dst_h = qTh[:, :, ts(qt_, P)]
        dst_l = qTl[:, :, ts(qt_, P)]
        view = pst[:].rearrange("p (o q) -> p o q", q=P)
        nc.scalar.copy(dst_h, view)
        nc.vector.tensor_tensor(dst_l, view, dst_h, AOP.subtract)
    return qTh, qTl


def _stage_q(nc, pools, b, q_d):
    stage, dram = pools["stage"], pools["dram"]
    qt_pool = pools["qt"]
    q_hi_scr = dram.tile([T, T], F16, tag="qhi")
    q_lo_scr = dram.tile([T, T], F16, tag="qlo")
    _hi_lo_to_scratch(nc, stage, q_d[b], q_hi_scr, q_lo_scr)
    qTh = qt_pool.tile([P, NT, T], F16, tag="qTh")
    qTl = qt_pool.tile([P, NT, T], F16, tag="qTl")
    for dt_ in range(NT):
        nc.sync.dma_start_transpose(qTh[:, dt_, :], q_hi_scr[:, ts(dt_, P)])
        nc.sync.dma_start_transpose(qTl[:, dt_, :], q_lo_scr[:, ts(dt_, P)])
    return qTh, qTl


def _stage_rest(nc, pools, b, tensors):
    q_d, k_d, v_d, m_d, s_d, c_d = tensors
    stage, small, dram = pools["stage"], pools["small"], pools["dram"]
    kt_pool = pools["kt"]

    # mask -> fp32 broadcast to all partitions
    mf = stage.tile([P, T], F32, tag="ldf32", bufs=4)
    _ring(nc, OPTS["stage_ring"]).dma_start(mf[:1, :], m_d[b : b + 1, :])
    mrep = small.tile([P, T], F32, tag="mrep")
    nc.gpsimd.partition_broadcast(mrep[:], mf[:1, :])

    k_hi_scr = dram.tile([T, T], F16, tag="khi")
    k_lo_scr = dram.tile([T, T], F16, tag="klo")
    with pools["tc"].tile_wait_until(OPTS["k_floor_ms"]):
        _hi_lo_to_scratch(nc, stage, k_d[b], k_hi_scr, k_lo_scr)
    kTh = kt_pool.tile([P, NT, T], F16, tag="kTh")
    kTl = kt_pool.tile([P, NT, T], F16, tag="kTl")
    for dt_ in range(NT):
        nc.sync.dma_start_transpose(kTh[:, dt_, :], k_hi_scr[:, ts(dt_, P)])
        nc.sync.dma_start_transpose(kTl[:, dt_, :], k_lo_scr[:, ts(dt_, P)])
    return kTh, kTl, mrep


def _stage_batch(nc, pools, b, tensors):
    qTh, qTl = _stage_q(nc, pools, b, tensors[0])
    kTh, kTl, mrep = _stage_rest(nc, pools, b, tensors)
    return qTh, qTl, kTh, kTl, mrep


def _load_values(nc, pools, b, v_d, not_before_ms=0.0):
    vals = pools["val"].tile([P, NT, T], F16, tag="vals")
    tc = pools["tc"]
    # SWDGE cast-load: fp32 DRAM -> fp16 SBUF, no compute engine involved.
    # not_before keeps the scheduler from hoisting these 4MB loads into the
    # startup window where DMA bandwidth is the critical resource.
    with tc.tile_wait_until(not_before_ms):
        for kt_ in range(NT):
            nc.gpsimd.dma_start(vals[:, kt_, :], v_d[b, ts(kt_, P), :])
    return vals


PHASE_MARKS = []


def _mark(nc, label):
    PHASE_MARKS.append((int(nc.next_id()), label))


def build_nc(reps=1):
    PHASE_MARKS.clear()
    nc = bacc.Bacc("TRN2", target_bir_lowering=False, debug=False, num_devices=NCORES)
    q_d = nc.dram_tensor("query", [NB, T, T], F32, kind="ExternalInput")
    k_d = nc.dram_tensor("keys", [NB, T, T], F32, kind="ExternalInput")
    v_d = nc.dram_tensor("values", [NB, T, T], F32, kind="ExternalInput")
    w_d = nc.dram_tensor("W", [T, T], F32, kind="ExternalInput")
    m_d = nc.dram_tensor("mask", [NB, T], F32, kind="ExternalInput")
    s_d = nc.dram_tensor("score", [NB, T, T], F32, kind="ExternalOutput")
    c_d = nc.dram_tensor("ctx", [NB, T, T], F32, kind="ExternalOutput")

    with tile.TileContext(nc) as tc:
        with (
            tc.tile_pool(name="stage", bufs=2) as stage,
            tc.tile_pool(name="wres", bufs=1) as wres,
            tc.tile_pool(name="qt", bufs=1) as qt_pool,
            tc.tile_pool(name="qwt", bufs=1) as qwt_pool,
            tc.tile_pool(name="kt", bufs=1) as kt_pool,
            tc.tile_pool(name="val", bufs=1) as val_pool,
            tc.tile_pool(name="soft", bufs=2) as soft,
            tc.tile_pool(name="sc", bufs=2) as sc_pool,
            tc.tile_pool(name="st", bufs=2) as st_pool,
            tc.tile_pool(name="cx", bufs=1) as cx_pool,
            tc.tile_pool(name="small", bufs=1) as small,
            tc.tile_pool(name="ones", bufs=1) as ones_pool,
        ):
            with (
                tc.tile_pool(name="psA", bufs=2, space="PSUM") as psA,
                tc.tile_pool(name="psB", bufs=2, space="PSUM") as psB,
                tc.tile_pool(name="dram", bufs=2, space="DRAM") as dram,
            ):
                pools = {
                    "tc": tc, "s16_tiles": {}, "ident16": None,
                    "stage": stage, "qt": qt_pool, "qwt": qwt_pool,
                    "kt": kt_pool, "val": val_pool, "soft": soft, "sc": sc_pool,
                    "st": st_pool, "cx": cx_pool, "small": small,
                    "psA": psA, "psB": psB, "dram": dram,
                }
                ones = ones_pool.tile([P, P], F16)
                nc.vector.memset(ones[:], 1.0 / P)
                ident = ones_pool.tile([P, P], F32, tag="ident")
                ident16 = ones_pool.tile([P, P], F16, tag="ident16")
                from concourse.masks import make_identity
                make_identity(nc, ident[:])
                make_identity(nc, ident16[:])
                pools["ident16"] = ident16

                tensors = (q_d, k_d, v_d, m_d, s_d, c_d)
                if OPTS["q0_pe_t"]:
                    qT0 = _stage_q_pe(nc, pools, 0, q_d, ident)
                else:
                    qT0 = _stage_q(nc, pools, 0, q_d)

                # W -> hi/lo fp16, resident in SBUF (row layout = lhsT layout)
                W_hi = wres.tile([P, NT, T], F16, tag="Whi")
                W_lo = wres.tile([P, NT, T], F16, tag="Wlo")
                for dt_ in range(NT):
                    wf = stage.tile([P, T], F32, tag="ldf32", bufs=4)
                    _ring(nc, OPTS["load_ring"]).dma_start(wf[:], w_d[ts(dt_, P), :])
                    nc.vector.tensor_copy(W_hi[:, dt_, :], wf[:])
                    nc.vector.tensor_tensor(W_lo[:, dt_, :], wf[:], W_hi[:, dt_, :], AOP.subtract)

                for _rep in range(reps):
                    _mark(nc, "setupW-done")
                    if OPTS["order"] == "pipelined":
                        if _rep == 0:
                            st0 = qT0 + _stage_rest(nc, pools, 0, tensors)
                        else:
                            st0 = _stage_batch(nc, pools, 0, tensors)
                        _mark(nc, "stage0")
                        qWT0 = _phase1(nc, pools, 0, W_hi, W_lo, st0[0], st0[1])
                        _mark(nc, "p1b0")
                        s16_scr0 = dram.tile([T, T], F16, tag="s16")
                        if OPTS["interleave_both"]:
                            vals0 = _load_values(nc, pools, 0, v_d, not_before_ms=0.12)
                            _phase2_softmax(
                                nc, pools, 0, s_d, qWT0[0], qWT0[1],
                                st0[2], st0[3], ones, st0[4], s16_scr0,
                                interleave=lambda qt_, tail=False: _phase3_qt(
                                    nc, pools, 0, c_d, s16_scr0, vals0, qt_,
                                    ps_pool="psB" if tail else "psA",
                                ),
                                skip_tail=OPTS["p3b0_in_p1b1"],
                                off=OPTS.get("il_offset0"),
                            )
                        else:
                            _phase2_softmax(nc, pools, 0, s_d, qWT0[0], qWT0[1],
                                            st0[2], st0[3], ones, st0[4], s16_scr0)
                        _mark(nc, "p2b0")
                        st1 = _stage_batch(nc, pools, 1, tensors)
                        _mark(nc, "stage1")
                        off = OPTS["il_offset"]
                        if OPTS["p3b0_in_p1b1"] and OPTS["interleave_both"]:
                            p1il = lambda et: (
                                _phase3_qt(nc, pools, 0, c_d, s16_scr0, vals0,
                                           NT - off + et, ps_pool="psB")
                                if et < off else None
                            )
                        else:
                            p1il = None
                        qWT1 = _phase1(nc, pools, 1, W_hi, W_lo, st1[0], st1[1],
                                       interleave=p1il)
                        _mark(nc, "p1b1")
                        if not OPTS["interleave_both"]:
                            vals0 = _load_values(nc, pools, 0, v_d)
                        if not (OPTS["interleave_p3b0"] or OPTS["interleave_both"]):
                            _phase3(nc, pools, 0, c_d, s16_scr0, vals0)
                        _mark(nc, "p3b0")
                        s16_scr1 = dram.tile([T, T], F16, tag="s16")
                        vals1 = _load_values(nc, pools, 1, v_d, not_before_ms=0.3)
                        if OPTS["interleave_p3b0"]:
                            _phase2_softmax(
                                nc, pools, 1, s_d, qWT1[0], qWT1[1],
                                st1[2], st1[3], ones, st1[4], s16_scr1,
                                interleave=lambda qt_: _phase3_qt(
                                    nc, pools, 0, c_d, s16_scr0, vals0, qt_
                                ),
                            )
                            _phase3(nc, pools, 1, c_d, s16_scr1, vals1)
                        elif OPTS["interleave_p3b1"]:
                            _phase2_softmax(
                                nc, pools, 1, s_d, qWT1[0], qWT1[1],
                                st1[2], st1[3], ones, st1[4], s16_scr1,
                                interleave=lambda qt_, tail=False: (
                                    _phase3_qt_pe(
                                        nc, pools, 1, c_d,
                                        pools["s16_tiles"][(1, qt_)], vals1, qt_,
                                    )
                                    if tail and qt_ >= NT - 2
                                    else _phase3_qt(
                                        nc, pools, 1, c_d, s16_scr1, vals1, qt_,
                                        ps_pool="psB" if tail else "psA",
                                    )
                                ),
                            )
                        else:
                            _phase2_softmax(nc, pools, 1, s_d, qWT1[0], qWT1[1],
                                            st1[2], st1[3], ones, st1[4], s16_scr1)
                            _phase3(nc, pools, 1, c_d, s16_scr1, vals1)
                        _mark(nc, "p2b1+p3b1")
                    else:
                        for b in range(NB):
                            stb = _stage_batch(nc, pools, b, tensors)
                            _mark(nc, f"stage{b}")
                            qWTb = _phase1(nc, pools, b, W_hi, W_lo, stb[0], stb[1])
                            _mark(nc, f"p1b{b}")
                            s16_scrb = dram.tile([T, T], F16, tag="s16")
                            _phase2_softmax(nc, pools, b, s_d, qWTb[0], qWTb[1],
                                            stb[2], stb[3], ones, stb[4], s16_scrb)
                            _mark(nc, f"p2b{b}")
                            valsb = _load_values(nc, pools, b, v_d)
                            _phase3(nc, pools, b, c_d, s16_scrb, valsb)
                            _mark(nc, f"p3b{b}")

    nc.compile()
    return nc


_nc = None


def _get_nc():
    global _nc
    if _nc is None:
        _nc = build_nc()
    return _nc


def make_in_maps(query, keys, values, W, mask):
    query = np.ascontiguousarray(np.asarray(query, dtype=np.float32))
    keys = np.ascontiguousarray(np.asarray(keys, dtype=np.float32))
    values = np.ascontiguousarray(np.asarray(values, dtype=np.float32))
    W = np.ascontiguousarray(np.asarray(W, dtype=np.float32))
    mask = np.ascontiguousarray(np.asarray(mask, dtype=np.float32))
    in_maps = []
    for c in range(NCORES):
        sl = slice(c * NB, (c + 1) * NB)
        in_maps.append(
            {
                "query": query[sl],
                "keys": keys[sl],
                "values": values[sl],
                "W": W,
                "mask": mask[sl],
            }
        )
    return in_maps


def kernel(query, keys, values, W, mask):
    nc = _get_nc()
    in_maps = make_in_maps(query, keys, values, W, mask)
    res = run_bass_kernel_spmd(nc, in_maps, core_ids=list(range(NCORES)))
    score = np.concatenate([res.results[c]["score"] for c in range(NCORES)], axis=0)
    ctx = np.concatenate([res.results[c]["ctx"] for c in range(NCORES)], axis=0)
    return score, ctx


# revision 64
# speedup vs baseline: 1.0222x; 1.0222x over previous
"""Bass/Tile TRN2 kernel for bilinear-score attention (score softmax + context).

reference:
    qW     = query @ W                      [B, Tq, Dk]
    weight = qW @ keys^T + mask[:, None, :] [B, Tq, Tk]
    score  = softmax(weight, axis=-1)
    ctx    = score @ values                 [B, Tq, Dv]
    returns (score, ctx)

Sharding: data-parallel over batch B=16 across 8 NeuronCores (2 batches/core).
Numerics: fp16 hi/lo 3-pass matmuls (hh + hl + lh) for both big contractions
(near-fp32 logits; exact fp32 mask added during the PSUM drain); phase 3 is a
single all-fp16 pass (softmax scores and values cast to fp16, values cast
in-flight by the SWDGE DMA). Measured vs fp32 reference: score rel err ~4e-5,
ctx rel err ~5e-4. Per-core device time ~439 us (cost-model timeline, ~93% PE occupancy;
corroborated by an isolated-cache in-NEFF-repetition wall-clock slope).
"""

import os
import sys

import numpy as np

os.environ.setdefault("JAX_COMPILATION_CACHE_DIR", "/tmp/jax_comp_cache")

for _p in ("/opt/trn_rl_repo",):
    if _p not in sys.path and os.path.isdir(_p):
        sys.path.insert(0, _p)

import concourse.bass as bass  # noqa: E402
import concourse.tile as tile  # noqa: E402
from concourse import bacc, mybir  # noqa: E402
from concourse.bass import ds, ts  # noqa: E402
from concourse.bass_utils import run_bass_kernel_spmd  # noqa: E402

import json as _json

OPTS = {
    "drain_copy": True,      # early PSUM->SBUF drain, softmax reads copy
    "vals_swdge": False,     # values load via gpsimd SWDGE
    "stage_hi": "act",       # engine for hi cast: act|dve|gpsimd
    "stage_lo": "dve",       # engine for lo subtract: dve|gpsimd
    "stage_ring": "sync",    # ring for staging dma: sync|scalar
    "order": "pipelined",    # pipelined|serial
    "interleave_p3b1": True,
    "interleave_p3b0": False,
    "interleave_both": True,
    "score_t": "perqt",      # perqt|batched
    "out_ring": "sync",      # ring for score/ctx/s16 outputs
    "pair_stage": False,     # batch staging loads/writes in pairs
    "s16_ring": "gpsimd",
    "load_ring": "sync",
    "il_offset": 4,
    "stagew_split": False,
    "p3b0_in_p1b1": True,
    "q0_pe_t": True,
    "k_floor_ms": 0.03,
}
if os.environ.get("K_OPTS"):
    OPTS.update(_json.loads(os.environ["K_OPTS"]))

P = 128
T = 1024
NT = T // P  # 8
NB = 2       # batches per core
NCORES = 8
F32 = mybir.dt.float32
F16 = mybir.dt.float16
F32R = mybir.dt.float32r
AX = mybir.AxisListType
AOP = mybir.AluOpType
AF = mybir.ActivationFunctionType


def _eng(nc, name):
    return {"act": nc.scalar, "dve": nc.vector, "gpsimd": nc.gpsimd}[name]


def _ring(nc, name):
    return {"sync": nc.sync, "scalar": nc.scalar, "gpsimd": nc.gpsimd}[name]


def _hi_lo_to_scratch(nc, stage, src_ap, hi_scr, lo_scr):
    """Load fp32 rows, split into fp16 hi/lo, store to DRAM scratch."""
    ring = _ring(nc, OPTS["stage_ring"])
    hi_e = _eng(nc, OPTS["stage_hi"])
    lo_e = _eng(nc, OPTS["stage_lo"])
    if not OPTS["pair_stage"]:
        for t in range(NT):
            xf = stage.tile([P, T], F32, tag="ldf32", bufs=4)
            _ring(nc, OPTS["load_ring"]).dma_start(xf[:], src_ap[ts(t, P), :])
            xh = stage.tile([P, T], F16, tag="hi16", bufs=4)
            if hi_e is nc.scalar:
                nc.scalar.copy(xh[:], xf[:])
            else:
                hi_e.tensor_copy(xh[:], xf[:])
            xl = stage.tile([P, T], F16, tag="lo16")
            lo_e.tensor_tensor(xl[:], xf[:], xh[:], AOP.subtract)
            ring.dma_start(hi_scr[ts(t, P), :], xh[:])
            ring.dma_start(lo_scr[ts(t, P), :], xl[:])
        return
    for t2 in range(NT // 2):
        xf = stage.tile([P, 2, T], F32, tag="ldf32pair", bufs=1)
        ring.dma_start(xf[:], src_ap[ds(t2 * 2 * P, 2 * P), :].rearrange("(o p) e -> p o e", p=P))
        xh = stage.tile([P, 2, T], F16, tag="hi16")
        xl = stage.tile([P, 2, T], F16, tag="lo16")
        for j in range(2):
            if hi_e is nc.scalar:
                nc.scalar.copy(xh[:, j, :], xf[:, j, :])
            else:
                hi_e.tensor_copy(xh[:, j, :], xf[:, j, :])
            lo_e.tensor_tensor(xl[:, j, :], xf[:, j, :], xh[:, j, :], AOP.subtract)
        ring.dma_start(
            hi_scr[ds(t2 * 2 * P, 2 * P), :].rearrange("(o p) e -> p o e", p=P), xh[:]
        )
        ring.dma_start(
            lo_scr[ds(t2 * 2 * P, 2 * P), :].rearrange("(o p) e -> p o e", p=P), xl[:]
        )


def _phase1(nc, pools, b, W_hi, W_lo, qTh, qTl, interleave=None):
    """qWT[e, q] = W^T @ query^T as fp16 hi/lo, 3-pass per psum tile."""
    qwt_pool, psA = pools["qwt"], pools["psA"]
    qWTh = qwt_pool.tile([P, NT, T], F16, tag="qWTh")
    qWTl = qwt_pool.tile([P, NT, T], F16, tag="qWTl")
    for et in range(NT):
        if interleave is not None:
            interleave(et)
        ps = psA.tile([P, T], F32, tag="psA")
        for dt_ in range(NT):
            lw_h = W_hi[:, dt_, ts(et, P)]
            lw_l = W_lo[:, dt_, ts(et, P)]
            first = dt_ == 0
            last = dt_ == NT - 1
            for qc in range(2):
                nc.tensor.matmul(
                    ps[:, ds(qc * 512, 512)], lw_h,
                    qTh[:, dt_, ds(qc * 512, 512)], start=first, stop=False,
                )
            for qc in range(2):
                nc.tensor.matmul(
                    ps[:, ds(qc * 512, 512)], lw_h,
                    qTl[:, dt_, ds(qc * 512, 512)], start=False, stop=False,
                )
            for qc in range(2):
                nc.tensor.matmul(
                    ps[:, ds(qc * 512, 512)], lw_l,
                    qTh[:, dt_, ds(qc * 512, 512)], start=False, stop=last,
                )
        nc.scalar.copy(qWTh[:, et, :], ps[:])
        nc.vector.tensor_tensor(qWTl[:, et, :], ps[:], qWTh[:, et, :], AOP.subtract)
    return qWTh, qWTl


def _phase2_softmax(nc, pools, b, s_d, qWTh, qWTl, kTh, kTl, ones, mrep, s16_scr,
                    interleave=None, skip_tail=False, off=None):
    """weight[q, k] = qW @ keys^T + mask; softmax rows; write score + fp16 copy."""
    soft, sc_pool, psB = pools["soft"], pools["sc"], pools["psB"]
    if off is None:
        off = OPTS["il_offset"]
    for qt_ in range(NT):
        if interleave is not None and qt_ >= off:
            interleave(qt_ - off, False)
        ps2 = psB.tile([P, T], F32, tag="psB")
        for et in range(NT):
            for li, (lhs, rhs) in enumerate(((qWTh, kTh), (qWTh, kTl), (qWTl, kTh))):
                lw = lhs[:, et, ts(qt_, P)]
                for kc in range(2):
                    nc.tensor.matmul(
                        ps2[:, ds(kc * 512, 512)], lw, rhs[:, et, ds(kc * 512, 512)],
                        start=(et == 0 and li == 0),
                        stop=(et == NT - 1 and li == 2),
                    )
        # drain + mask add fused: wsb = ps2 + mask (fp32, exact)
        wsb = soft.tile([P, T], F32, tag="wsb")
        nc.vector.tensor_tensor(wsb[:], ps2[:], mrep[:], AOP.add)
        negmax = soft.tile([P, 1], F32, tag="negmax")
        nc.vector.tensor_reduce(negmax[:], wsb[:], axis=AX.X, op=AOP.max, negate=True)
        expt = soft.tile([P, T], F32, tag="expt")
        sumexp = soft.tile([P, 1], F32, tag="sumexp")
        nc.scalar.activation(
            expt[:], wsb[:], AF.Exp, bias=negmax[:], scale=1.0, accum_out=sumexp[:]
        )
        recip = soft.tile([P, 1], F32, tag="recip")
        nc.vector.reciprocal(recip[:], sumexp[:])
        nc.vector.tensor_scalar_mul(expt[:], expt[:], recip[:])
        _ring(nc, OPTS["out_ring"]).dma_start(s_d[b, ts(qt_, P), :], expt[:])
        s16 = sc_pool.tile([P, T], F16, tag="s16t")
        pools["s16_tiles"][(b, qt_)] = s16
        nc.scalar.copy(s16[:], expt[:])
        _ring(nc, OPTS["s16_ring"]).dma_start(s16_scr[ts(qt_, P), :], s16[:])
    if interleave is not None and not skip_tail:
        for qt_ in range(NT - off, NT):
            interleave(qt_, True)


def _phase3_qt_pe(nc, pools, b, c_d, s16_tile, vals, qt_, ps_pool="psB"):
    """Tail variant: transpose score on the (idle) PE from the resident s16
    SBUF tile via an fp16 PSUM bitcast view; skips the scratch round-trip."""
    st_pool, cx_pool = pools["st"], pools["cx"]
    ident16 = pools["ident16"]
    ps = pools[ps_pool].tile([P, T], F32, tag=ps_pool)
    view = ps[:, ds(0, 512)].bitcast(F16)
    for kt_ in range(NT):
        nc.tensor.transpose(view[:, ts(kt_, P)], s16_tile[:, ts(kt_, P)], ident16)
    sT16 = st_pool.tile([P, NT, P], F16, tag="sT16")
    nc.scalar.copy(sT16[:], view.rearrange("p (o q) -> p o q", q=P))
    ps3 = ps[:, ds(512, 512)]
    # two vc halves accumulate sequentially into the same psum half (bank 1)
    cxt = cx_pool.tile([P, T], F32, tag="cx")
    for vc in range(2):
        for kt_ in range(NT):
            nc.tensor.matmul(
                ps3, sT16[:, kt_, :], vals[:, kt_, ds(vc * 512, 512)],
                start=(kt_ == 0), stop=(kt_ == NT - 1),
            )
        nc.scalar.copy(cxt[:, ds(vc * 512, 512)], ps3)
    _ring(nc, OPTS["out_ring"]).dma_start(c_d[b, ts(qt_, P), :], cxt[:])


def _phase3_qt(nc, pools, b, c_d, s16_scr, vals, qt_, ps_pool="psA"):
    """ctx[qt block] = score @ values, all fp16, one pass."""
    st_pool, cx_pool = pools["st"], pools["cx"]
    sT16 = st_pool.tile([P, NT, P], F16, tag="sT16")
    for kt_ in range(NT):
        nc.sync.dma_start_transpose(sT16[:, kt_, :], s16_scr[ts(qt_, P), ts(kt_, P)])
    ps3 = pools[ps_pool].tile([P, T], F32, tag=ps_pool)
    for kt_ in range(NT):
        lw = sT16[:, kt_, :]
        for vc in range(2):
            nc.tensor.matmul(
                ps3[:, ds(vc * 512, 512)], lw, vals[:, kt_, ds(vc * 512, 512)],
                start=(kt_ == 0), stop=(kt_ == NT - 1),
            )
    cx = cx_pool.tile([P, T], F32, tag="cx")
    nc.scalar.copy(cx[:], ps3[:])
    _ring(nc, OPTS["out_ring"]).dma_start(c_d[b, ts(qt_, P), :], cx[:])


def _phase3(nc, pools, b, c_d, s16_scr, vals):
    if OPTS["score_t"] == "perqt":
        for qt_ in range(NT):
            _phase3_qt(nc, pools, b, c_d, s16_scr, vals, qt_)
        return
    st_pool, cx_pool, psA = pools["st"], pools["cx"], pools["psA"]
    sTall = st_pool.tile([P, NT, T], F16, tag="sTall", bufs=1)
    for kt_ in range(NT):
        nc.sync.dma_start_transpose(sTall[:, kt_, :], s16_scr[:, ts(kt_, P)])
    for qt_ in range(NT):
        ps3 = psA.tile([P, T], F32, tag="psA")
        for kt_ in range(NT):
            lw = sTall[:, kt_, ts(qt_, P)]
            for vc in range(2):
                nc.tensor.matmul(
                    ps3[:, ds(vc * 512, 512)], lw, vals[:, kt_, ds(vc * 512, 512)],
                    start=(kt_ == 0), stop=(kt_ == NT - 1),
                )
        cx = cx_pool.tile([P, T], F32, tag="cx")
        nc.scalar.copy(cx[:], ps3[:])
        _ring(nc, OPTS["out_ring"]).dma_start(c_d[b, ts(qt_, P), :], cx[:])


def _stage_q_pe(nc, pools, b, q_d, ident):
    """Transpose query on the (idle) PE: fp32 tile transposes into PSUM, then
    one fused drain per q-tile into the fp16 hi/lo qT layout. No DRAM
    round-trip, no xbar."""
    stage, qt_pool, psB = pools["stage"], pools["qt"], pools["psB"]
    qTh = qt_pool.tile([P, NT, T], F16, tag="qTh")
    qTl = qt_pool.tile([P, NT, T], F16, tag="qTl")
    for qt_ in range(NT):
        xf = stage.tile([P, T], F32, tag="ldf32", bufs=4)
        _ring(nc, OPTS["load_ring"]).dma_start(xf[:], q_d[b, ts(qt_, P), :])
        pst = psB.tile([P, T], F32, tag="psB")
        for dt_ in range(NT):
            nc.tensor.transpose(pst[:, ts(dt_, P)], xf[:, ts(dt_, P)], ident)
        dst_h = qTh[:, :, ts(qt_, P)]
        dst_l = qTl[:, :, ts(qt_, P)]
        view = pst[:].rearrange("p (o q) -> p o q", q=P)
        nc.scalar.copy(dst_h, view)
        nc.vector.tensor_tensor(dst_l, view, dst_h, AOP.subtract)
    return qTh, qTl


def _stage_q(nc, pools, b, q_d):
    stage, dram = pools["stage"], pools["dram"]
    qt_pool = pools["qt"]
    q_hi_scr = dram.tile([T, T], F16, tag="qhi")
    q_lo_scr = dram.tile([T, T], F16, tag="qlo")
    _hi_lo_to_scratch(nc, stage, q_d[b], q_hi_scr, q_lo_scr)
    qTh = qt_pool.tile([P, NT, T], F16, tag="qTh")
    qTl = qt_pool.tile([P, NT, T], F16, tag="qTl")
    for dt_ in range(NT):
        nc.sync.dma_start_transpose(qTh[:, dt_, :], q_hi_scr[:, ts(dt_, P)])
        nc.sync.dma_start_transpose(qTl[:, dt_, :], q_lo_scr[:, ts(dt_, P)])
    return qTh, qTl


def _stage_rest(nc, pools, b, tensors):
    q_d, k_d, v_d, m_d, s_d, c_d = tensors
    stage, small, dram = pools["stage"], pools["small"], pools["dram"]
    kt_pool = pools["kt"]

    # mask -> fp32 broadcast to all partitions
    mf = stage.tile([P, T], F32, tag="ldf32", bufs=4)
    _ring(nc, OPTS["stage_ring"]).dma_start(mf[:1, :], m_d[b : b + 1, :])
    mrep = small.tile([P, T], F32, tag="mrep")
    nc.gpsimd.partition_broadcast(mrep[:], mf[:1, :])

    k_hi_scr = dram.tile([T, T], F16, tag="khi")
    k_lo_scr = dram.tile([T, T], F16, tag="klo")
    with pools["tc"].tile_wait_until(OPTS["k_floor_ms"]):
        _hi_lo_to_scratch(nc, stage, k_d[b], k_hi_scr, k_lo_scr)
    kTh = kt_pool.tile([P, NT, T], F16, tag="kTh")
    kTl = kt_pool.tile([P, NT, T], F16, tag="kTl")
    for dt_ in range(NT):
        nc.sync.dma_start_transpose(kTh[:, dt_, :], k_hi_scr[:, ts(dt_, P)])
        nc.sync.dma_start_transpose(kTl[:, dt_, :], k_lo_scr[:, ts(dt_, P)])
    return kTh, kTl, mrep


def _stage_batch(nc, pools, b, tensors):
    qTh, qTl = _stage_q(nc, pools, b, tensors[0])
    kTh, kTl, mrep = _stage_rest(nc, pools, b, tensors)
    return qTh, qTl, kTh, kTl, mrep


def _load_values(nc, pools, b, v_d, not_before_ms=0.0):
    vals = pools["val"].tile([P, NT, T], F16, tag="vals")
    tc = pools["tc"]
    # SWDGE cast-load: fp32 DRAM -> fp16 SBUF, no compute engine involved.
    # not_before keeps the scheduler from hoisting these 4MB loads into the
    # startup window where DMA bandwidth is the critical resource.
    with tc.tile_wait_until(not_before_ms):
        for kt_ in range(NT):
            nc.gpsimd.dma_start(vals[:, kt_, :], v_d[b, ts(kt_, P), :])
    return vals


PHASE_MARKS = []


def _mark(nc, label):
    PHASE_MARKS.append((int(nc.next_id()), label))


def build_nc(reps=1):
    PHASE_MARKS.clear()
    nc = bacc.Bacc("TRN2", target_bir_lowering=False, debug=False, num_devices=NCORES)
    q_d = nc.dram_tensor("query", [NB, T, T], F32, kind="ExternalInput")
    k_d = nc.dram_tensor("keys", [NB, T, T], F32, kind="ExternalInput")
    v_d = nc.dram_tensor("values", [NB, T, T], F32, kind="ExternalInput")
    w_d = nc.dram_tensor("W", [T, T], F32, kind="ExternalInput")
    m_d = nc.dram_tensor("mask", [NB, T], F32, kind="ExternalInput")
    s_d = nc.dram_tensor("score", [NB, T, T], F32, kind="ExternalOutput")
    c_d = nc.dram_tensor("ctx", [NB, T, T], F32, kind="ExternalOutput")

    with tile.TileContext(nc) as tc:
        with (
            tc.tile_pool(name="stage", bufs=2) as stage,
            tc.tile_pool(name="wres", bufs=1) as wres,
            tc.tile_pool(name="qt", bufs=1) as qt_pool,
            tc.tile_pool(name="qwt", bufs=1) as qwt_pool,
            tc.tile_pool(name="kt", bufs=1) as kt_pool,
            tc.tile_pool(name="val", bufs=1) as val_pool,
            tc.tile_pool(name="soft", bufs=2) as soft,
            tc.tile_pool(name="sc", bufs=2) as sc_pool,
            tc.tile_pool(name="st", bufs=2) as st_pool,
            tc.tile_pool(name="cx", bufs=1) as cx_pool,
            tc.tile_pool(name="small", bufs=1) as small,
            tc.tile_pool(name="ones", bufs=1) as ones_pool,
        ):
            with (
                tc.tile_pool(name="psA", bufs=2, space="PSUM") as psA,
                tc.tile_pool(name="psB", bufs=2, space="PSUM") as psB,
                tc.tile_pool(name="dram", bufs=2, space="DRAM") as dram,
            ):
                pools = {
                    "tc": tc, "s16_tiles": {}, "ident16": None,
                    "stage": stage, "qt": qt_pool, "qwt": qwt_pool,
                    "kt": kt_pool, "val": val_pool, "soft": soft, "sc": sc_pool,
                    "st": st_pool, "cx": cx_pool, "small": small,
                    "psA": psA, "psB": psB, "dram": dram,
                }
                ones = ones_pool.tile([P, P], F16)
                nc.vector.memset(ones[:], 1.0 / P)
                ident = ones_pool.tile([P, P], F32, tag="ident")
                ident16 = ones_pool.tile([P, P], F16, tag="ident16")
                from concourse.masks import make_identity
                make_identity(nc, ident[:])
                make_identity(nc, ident16[:])
                pools["ident16"] = ident16

                tensors = (q_d, k_d, v_d, m_d, s_d, c_d)
                if OPTS["q0_pe_t"]:
                    qT0 = _stage_q_pe(nc, pools, 0, q_d, ident)
                else:
                    qT0 = _stage_q(nc, pools, 0, q_d)

                # W -> hi/lo fp16, resident in SBUF (row layout = lhsT layout)
                W_hi = wres.tile([P, NT, T], F16, tag="Whi")
                W_lo = wres.tile([P, NT, T], F16, tag="Wlo")
                for dt_ in range(NT):
                    wf = stage.tile([P, T], F32, tag="ldf32", bufs=4)
                    _ring(nc, OPTS["load_ring"]).dma_start(wf[:], w_d[ts(dt_, P), :])
                    nc.vector.tensor_copy(W_hi[:, dt_, :], wf[:])
                    nc.vector.tensor_tensor(W_lo[:, dt_, :], wf[:], W_hi[:, dt_, :], AOP.subtract)

                for _rep in range(reps):
                    _mark(nc, "setupW-done")
                    if OPTS["order"] == "pipelined":
                        if _rep == 0:
                            st0 = qT0 + _stage_rest(nc, pools, 0, tensors)
                        else:
                            st0 = _stage_batch(nc, pools, 0, tensors)
                        _mark(nc, "stage0")
                        qWT0 = _phase1(nc, pools, 0, W_hi, W_lo, st0[0], st0[1])
                        _mark(nc, "p1b0")
                        s16_scr0 = dram.tile([T, T], F16, tag="s16")
                        if OPTS["interleave_both"]:
                            vals0 = _load_values(nc, pools, 0, v_d, not_before_ms=0.12)
                            _phase2_softmax(
                                nc, pools, 0, s_d, qWT0[0], qWT0[1],
                                st0[2], st0[3], ones, st0[4], s16_scr0,
                                interleave=lambda qt_, tail=False: _phase3_qt(
                                    nc, pools, 0, c_d, s16_scr0, vals0, qt_,
                                    ps_pool="psB" if tail else "psA",
                                ),
                                skip_tail=OPTS["p3b0_in_p1b1"],
                                off=OPTS.get("il_offset0"),
                            )
                        else:
                            _phase2_softmax(nc, pools, 0, s_d, qWT0[0], qWT0[1],
                                            st0[2], st0[3], ones, st0[4], s16_scr0)
                        _mark(nc, "p2b0")
                        st1 = _stage_batch(nc, pools, 1, tensors)
                        _mark(nc, "stage1")
                        off = OPTS["il_offset"]
                        if OPTS["p3b0_in_p1b1"] and OPTS["interleave_both"]:
                            p1il = lambda et: (
                                _phase3_qt(nc, pools, 0, c_d, s16_scr0, vals0,
                                           NT - off + et // 2, ps_pool="psB")
                                if et % 2 == 0 and et // 2 < off else None
                            )
                        else:
                            p1il = None
                        qWT1 = _phase1(nc, pools, 1, W_hi, W_lo, st1[0], st1[1],
                                       interleave=p1il)
                        _mark(nc, "p1b1")
                        if not OPTS["interleave_both"]:
                            vals0 = _load_values(nc, pools, 0, v_d)
                        if not (OPTS["interleave_p3b0"] or OPTS["interleave_both"]):
                            _phase3(nc, pools, 0, c_d, s16_scr0, vals0)
                        _mark(nc, "p3b0")
                        s16_scr1 = dram.tile([T, T], F16, tag="s16")
                        vals1 = _load_values(nc, pools, 1, v_d, not_before_ms=0.3)
                        if OPTS["interleave_p3b0"]:
                            _phase2_softmax(
                                nc, pools, 1, s_d, qWT1[0], qWT1[1],
                                st1[2], st1[3], ones, st1[4], s16_scr1,
                                interleave=lambda qt_: _phase3_qt(
                                    nc, pools, 0, c_d, s16_scr0, vals0, qt_
                                ),
                            )
                            _phase3(nc, pools, 1, c_d, s16_scr1, vals1)
                        elif OPTS["interleave_p3b1"]:
                            _phase2_softmax(
                                nc, pools, 1, s_d, qWT1[0], qWT1[1],
                                st1[2], st1[3], ones, st1[4], s16_scr1,
                                interleave=lambda qt_, tail=False: (
                                    _phase3_qt_pe(
                                        nc, pools, 1, c_d,
                                        pools["s16_tiles"][(1, qt_)], vals1, qt_,
                                    )
                                    if tail and qt_ >= NT - 2
                                    else _phase3_qt(
                                        nc, pools, 1, c_d, s16_scr1, vals1, qt_,
                                        ps_pool="psB" if tail else "psA",
                                    )
                                ),
                            )
                        else:
                            _phase2_softmax(nc, pools, 1, s_d, qWT1[0], qWT1[1],
                                            st1[2], st1[3], ones, st1[4], s16_scr1)
                            _phase3(nc, pools, 1, c_d, s16_scr1, vals1)
                        _mark(nc, "p2b1+p3b1")
                    else:
                        for b in range(NB):
                            stb = _stage_batch(nc, pools, b, tensors)
                            _mark(nc, f"stage{b}")
                            qWTb = _phase1(nc, pools, b, W_hi, W_lo, stb[0], stb[1])
                            _mark(nc, f"p1b{b}")
                            s16_scrb = dram.tile([T, T], F16, tag="s16")
                            _phase2_softmax(nc, pools, b, s_d, qWTb[0], qWTb[1],
                                            stb[2], stb[3], ones, stb[4], s16_scrb)
                            _mark(nc, f"p2b{b}")
                            valsb = _load_values(nc, pools, b, v_d)
                            _phase3(nc, pools, b, c_d, s16_scrb, valsb)
                            _mark(nc, f"p3b{b}")

    nc.compile()
    return nc


_nc = None


def _get_nc():
    global _nc
    if _nc is None:
        _nc = build_nc()
    return _nc


def make_in_maps(query, keys, values, W, mask):
    query = np.ascontiguousarray(np.asarray(query, dtype=np.float32))
    keys = np.ascontiguousarray(np.asarray(keys, dtype=np.float32))
    values = np.ascontiguousarray(np.asarray(values, dtype=np.float32))
    W = np.ascontiguousarray(np.asarray(W, dtype=np.float32))
    mask = np.ascontiguousarray(np.asarray(mask, dtype=np.float32))
    in_maps = []
    for c in range(NCORES):
        sl = slice(c * NB, (c + 1) * NB)
        in_maps.append(
            {
                "query": query[sl],
                "keys": keys[sl],
                "values": values[sl],
                "W": W,
                "mask": mask[sl],
            }
        )
    return in_maps


def kernel(query, keys, values, W, mask):
    nc = _get_nc()
    in_maps = make_in_maps(query, keys, values, W, mask)
    res = run_bass_kernel_spmd(nc, in_maps, core_ids=list(range(NCORES)))
    score = np.concatenate([res.results[c]["score"] for c in range(NCORES)], axis=0)
    ctx = np.concatenate([res.results[c]["ctx"] for c in range(NCORES)], axis=0)
    return score, ctx
